# revision 14
# baseline (speedup 1.0000x reference)
"""Trainium2 Bass kernel for sorted segment_max (ClusterPool).

Problem: features [2M, 128] f32, segment_ids [2M] sorted int, num_clusters
10000 -> output [10000, 128] = per-cluster max over rows.

Strategy (8 NeuronCores, SPMD):
  - Shard rows: core c gets rows [c*250k, (c+1)*250k).  Sorted ids => each
    core covers a contiguous cluster range (~1252 clusters), padded to 1280
    local clusters = 10 batches x 128.
  - MAIN pass (aligned blocks): cluster rows are covered by the 8-row-aligned
    4KB blocks fully inside [s, e).  dma_gather with elem_step=1024 makes the
    int16 index a BLOCK index (RPC/8 = 31250 < 2^15), so the whole core is
    addressable with no window.  That allows batches to be ANY 128 clusters:
    we sort clusters by block count so every batch has near-uniform count,
    cutting padding from ~1.26x to ~1.03x.  Per batch of 128 clusters
    (cluster p on partition p), NH gather calls of <=8 blocks/cluster;
    tensor_reduce max over rows -> [128, 128]; tensor_tensor-combine; store.
  - BOUNDARY pass: the <=14 unaligned edge rows per cluster (head [s, 8*ceil(
    s/8)), tail [8*floor(e/8), e); all rows if no full block, provably <=14)
    are gathered row-granular (elem 512B) in CONSECUTIVE cluster batches
    (which fit the 32768-row int16 window), reduced to [128, 128], stored
    separately.
  - Host combines main partials (sorted order), boundary partials, across the
    8 cores with np.maximum; empty clusters -> -inf.
"""

import os
import sys

import numpy as np

sys.path.insert(0, "/opt/trn_rl_repo")

N_POINTS = 2_000_000
D = 128
N_CLUSTERS = 10_000
N_CORES = 8
RPC = N_POINTS // N_CORES  # rows per core
NBLKS = RPC // 8  # 8-row blocks per core (31250 < 2^15)
NCL = 1280  # padded local clusters per core
NBATCH = NCL // 128
WINDOW = 32768  # row window for the boundary gather
ELEM = 1024  # floats per main-gather element (8 rows)
NB = 14  # boundary row slots per cluster (max possible)
THMAX = 4  # max blocks per cluster per gather call

_last_results = None  # BassKernelResults of the most recent run (for test.py)


def _apply_drain_patch():
    """walrus TPB_CTRL supports a single sync wait; TileContext's tail drain
    accumulates one wait per outstanding proc.  Split them across NOPs."""
    import concourse.mybir as mybir
    import concourse.tile as tile
    from concourse.vector_clock import ScopedClock

    if getattr(tile.TileContext, "_drain_patched", False):
        return

    def _patched(self, tick_clock, wait_clock):
        nc = self.nc
        nop = nc.sync.nop(nofuse=True, hint="tail_drain_waits")
        wait_clock.add_sem_waits(nop.ins, ScopedClock({None: tick_clock.global_clock}))
        si = nop.ins.sync_info
        waits = list(si.on_wait) if si is not None and si.on_wait else []
        if len(waits) > 1:
            si.on_wait = waits[:1]
            for i in range(1, len(waits)):
                extra = nc.sync.nop(nofuse=True, hint=f"tail_drain_waits_{i}")
                if extra.ins.sync_info is None:
                    extra.ins.sync_info = mybir.SyncInfo(
                        on_wait=waits[i : i + 1], on_update=[]
                    )
                else:
                    extra.ins.sync_info.on_wait = waits[i : i + 1]
        nc.sync.drain()
        nc.all_engine_barrier()
        assert self.sems is not None
        popped = nc._tile_sem_poison_stack.pop()
        assert popped is self._sem_poison
        nc.clear_and_free_semaphores(list(self.sems.allocated().values()))
        nc.all_engine_barrier()

    tile.TileContext._drain_and_barrier = _patched
    tile.TileContext._drain_patched = True


def _build_program(tms, bwindows):
    """Build the SPMD Bass program.

    tms[m]      = main-pass blocks per cluster for sorted batch m
    bwindows[m] = boundary-pass window base row for consecutive batch m"""
    import concourse.bacc as bacc
    import concourse.mybir as mybir
    import concourse.tile as tile
    from concourse.bass import AP

    _apply_drain_patch()

    CM = max(tms) * 8  # main idx cols per batch table

    nc = bacc.Bacc(None, num_swdge_queues=4)
    f_in = nc.dram_tensor("features", [RPC, D], mybir.dt.float32, kind="ExternalInput")
    mi_in = nc.dram_tensor(
        "midx", [NBATCH, 128, CM], mybir.dt.int16, kind="ExternalInput"
    )
    bi_in = nc.dram_tensor(
        "bidx", [NBATCH, 128, NB * 8], mybir.dt.int16, kind="ExternalInput"
    )
    pm_out = nc.dram_tensor(
        "pmain", [NCL, D], mybir.dt.float32, kind="ExternalOutput"
    )
    pb_out = nc.dram_tensor(
        "pbnd", [NCL, D], mybir.dt.float32, kind="ExternalOutput"
    )

    fbase = f_in[:, :]
    blocks = fbase.rearrange("(a b) d -> a (b d)", b=8)  # [31250, 1024]
    gq = 0
    with tile.TileContext(nc) as tc:
        with (
            tc.tile_pool(name="gp", bufs=8) as gp,
            tc.tile_pool(name="bp", bufs=2) as bp,
            tc.tile_pool(name="ip", bufs=3) as ip,
            tc.tile_pool(name="sp", bufs=2) as sp,
        ):
            def boundary(m):
                # ---- boundary pass: consecutive batch m, edge rows -------
                w = bwindows[m]
                win = AP(fbase.tensor, w * D, [[D, WINDOW], [1, D]])
                bt = ip.tile([128, NB * 8], mybir.dt.int16, tag="bidx")
                nc.sync.dma_start(out=bt[:], in_=bi_in[m])
                bg = bp.tile([128, NB * D], mybir.dt.float32, tag="bg")
                nonlocal gq
                nc.gpsimd.dma_gather(
                    out_ap=bg[:].rearrange("p (t e) -> p t e", e=D),
                    in_ap=win,
                    idxs_ap=bt[:],
                    num_idxs=NB * 128,
                    num_idxs_reg=NB * 128,
                    elem_size=D,
                    queue_num=gq % 4,
                    single_packet=False,
                )
                gq += 1
                br = sp.tile([128, D], mybir.dt.float32, tag="bred")
                nc.vector.tensor_reduce(
                    out=br[:],
                    in_=bg[:].rearrange("p (t d) -> p d t", d=D),
                    axis=mybir.AxisListType.X,
                    op=mybir.AluOpType.max,
                )
                nc.sync.dma_start(out=pb_out[m * 128 : (m + 1) * 128, :], in_=br[:])

            for m in range(NBATCH):
                # ---- main pass: sorted batch m, aligned blocks -----------
                T = tms[m]
                it = ip.tile([128, CM], mybir.dt.int16, tag="midx")
                nc.sync.dma_start(out=it[:], in_=mi_in[m])
                # split the first batch's first call into a small primer so
                # the first reduce starts as early as possible
                splits = list(range(0, T, THMAX))
                chunks = [
                    (c0, min(THMAX, T - c0)) for c0 in splits
                ]
                if m == 0 and chunks[0][1] > 2:
                    chunks = [(0, 2), (2, chunks[0][1] - 2)] + chunks[1:]
                acc = None
                for c0, TH in chunks:
                    g = gp.tile([128, THMAX * ELEM], mybir.dt.float32, tag="gath")
                    nc.gpsimd.dma_gather(
                        out_ap=g[:, : TH * ELEM].rearrange(
                            "p (t e) -> p t e", e=ELEM
                        ),
                        in_ap=blocks,
                        idxs_ap=it[:, c0 * 8 : (c0 + TH) * 8],
                        num_idxs=TH * 128,
                        num_idxs_reg=TH * 128,
                        elem_size=ELEM,
                        queue_num=gq % 4,
                        single_packet=False,
                    )
                    gq += 1
                    r = sp.tile([128, D], mybir.dt.float32, tag=f"red{c0 % (2 * THMAX)}")
                    nc.vector.tensor_reduce(
                        out=r[:],
                        in_=g[:, : TH * ELEM].rearrange("p (t d) -> p d t", d=D),
                        axis=mybir.AxisListType.X,
                        op=mybir.AluOpType.max,
                    )
                    if acc is None:
                        acc = r
                    else:
                        nc.vector.tensor_tensor(
                            out=acc[:], in0=acc[:], in1=r[:],
                            op=mybir.AluOpType.max,
                        )
                nc.sync.dma_start(out=pm_out[m * 128 : (m + 1) * 128, :], in_=acc[:])
                boundary(m)

    if not nc.is_finalized():
        nc.finalize()
    return nc


def kernel(features, segment_ids, num_clusters):
    global _last_results
    from concourse.bass_utils import run_bass_kernel_spmd

    features = np.ascontiguousarray(np.asarray(features, dtype=np.float32))
    ids = np.asarray(segment_ids).astype(np.int64)
    nclusters = int(num_clusters)
    assert features.shape == (N_POINTS, D), features.shape
    assert ids.shape == (N_POINTS,)
    assert nclusters == N_CLUSTERS

    # --- host cluster metadata -------------------------------------------
    gstart = np.searchsorted(ids, np.arange(nclusters), side="left")
    gend = np.searchsorted(ids, np.arange(nclusters) + 1, side="left")
    gcounts = gend - gstart

    core_meta = []  # per core dict
    for c in range(N_CORES):
        r0, r1 = c * RPC, (c + 1) * RPC
        cl_lo, cl_hi = int(ids[r0]), int(ids[r1 - 1])
        ncl = cl_hi - cl_lo + 1
        assert ncl <= NCL, f"core {c}: {ncl} local clusters > {NCL}"
        s = np.clip(gstart[cl_lo : cl_hi + 1], r0, r1) - r0
        e = np.clip(gend[cl_lo : cl_hi + 1], r0, r1) - r0
        s_pad = np.zeros(NCL, dtype=np.int64)
        e_pad = np.zeros(NCL, dtype=np.int64)
        s_pad[:ncl] = s
        e_pad[:ncl] = e
        cnt = e_pad - s_pad
        a0 = (s_pad + 7) // 8  # first aligned block fully inside
        a1 = e_pad // 8  # one past last aligned block fully inside
        nb = np.maximum(a1 - a0, 0)
        nb[cnt == 0] = 0
        # cnt >= 15 guarantees nb >= 1; nb == 0 => cnt <= 14 (fits NB slots)
        assert int(cnt[(nb == 0)].max(initial=0)) <= NB
        order = np.argsort(nb, kind="stable")  # sorted batches for main pass
        core_meta.append(
            dict(cl_lo=cl_lo, ncl=ncl, s=s_pad, e=e_pad, cnt=cnt,
                 a0=a0, nb=nb, order=order)
        )

    # Main-pass per-batch block counts (shared across cores).
    tms = []
    for m in range(NBATCH):
        T = 1
        for cm in core_meta:
            sel = cm["order"][m * 128 : (m + 1) * 128]
            T = max(T, int(cm["nb"][sel].max()))
        tms.append(T)
    CM = max(tms) * 8

    # Boundary windows (consecutive batches, shared across cores).
    bwindows = []
    for m in range(NBATCH):
        jj = slice(m * 128, (m + 1) * 128)
        wmin = RPC
        for cm in core_meta:
            act = cm["cnt"][jj] > 0
            if act.any():
                wmin = min(wmin, int(cm["s"][jj][act].min()))
        w = max(0, min(wmin, RPC - WINDOW))
        bwindows.append(w)

    def wrap_calls(V, T):
        """V[j] with j = t*128+p for t in [0,T) -> wrapped int16 [128, T*8]."""
        tab = V.reshape(T * 8, 16).T  # [16, T*8]; col m covers V[m*16:(m+1)*16]
        return np.tile(tab, (8, 1))

    # --- main-pass tables -------------------------------------------------
    midx_all, bidx_all = [], []
    for cm in core_meta:
        g = np.zeros((NBATCH, 128, CM), dtype=np.int16)
        for m in range(NBATCH):
            T = tms[m]
            sel = cm["order"][m * 128 : (m + 1) * 128]
            a0 = cm["a0"][sel]
            nb = cm["nb"][sel]
            t = np.arange(T)[None, :]
            blk = a0[:, None] + t
            last = np.where(nb > 0, a0 + nb - 1, 0)
            blk = np.where(t >= nb[:, None], last[:, None], blk)
            blk = np.where((nb[:, None] > 0), blk, 0)
            assert blk.min() >= 0 and blk.max() < NBLKS
            V = blk.astype(np.int16).T.reshape(-1)  # j = t*128 + p
            g[m, :, : T * 8] = wrap_calls(V, T)
        midx_all.append(g)

        # boundary tables: consecutive batches
        b = np.zeros((NBATCH, 128, NB * 8), dtype=np.int16)
        for m in range(NBATCH):
            jj = slice(m * 128, (m + 1) * 128)
            s = cm["s"][jj]
            e = cm["e"][jj]
            cnt = cm["cnt"][jj]
            a0 = cm["a0"][jj]
            nb = cm["nb"][jj]
            w = bwindows[m]
            rows = np.zeros((128, NB), dtype=np.int64)
            for p in range(128):
                if cnt[p] == 0:
                    rows[p] = w
                    continue
                if nb[p] > 0:
                    head = np.arange(s[p], 8 * a0[p])
                    tail = np.arange(8 * (a0[p] + nb[p]), e[p])
                    rl = np.concatenate([head, tail])
                else:
                    rl = np.arange(s[p], e[p])
                nrl = len(rl)
                assert nrl <= NB
                rows[p, :nrl] = rl
                rows[p, nrl:] = s[p]  # repeat first row
            rel = rows - w
            assert rel.min() >= 0 and rel.max() < WINDOW, (m, rel.min(), rel.max())
            V = rel.astype(np.int16).T.reshape(-1)  # j = t*128+p, t in [0, NB)
            b[m] = wrap_calls(V, NB)
        bidx_all.append(b)

    # --- build + run ------------------------------------------------------
    nc = _build_program(tms, bwindows)
    in_maps = [
        {
            "features": features[c * RPC : (c + 1) * RPC],
            "midx": midx_all[c],
            "bidx": bidx_all[c],
        }
        for c in range(N_CORES)
    ]
    res = run_bass_kernel_spmd(nc, in_maps, list(range(N_CORES)))
    _last_results = res

    # --- host combine -----------------------------------------------------
    full = np.full((nclusters, D), -np.inf, dtype=np.float32)
    for c in range(N_CORES):
        cm = core_meta[c]
        cl_lo, ncl = cm["cl_lo"], cm["ncl"]
        pm = res.results[c]["pmain"]  # [NCL, D] in sorted order
        pb = res.results[c]["pbnd"]  # [NCL, D] in consecutive order
        order = cm["order"]
        nb_sorted = cm["nb"][order]
        valid_m = (nb_sorted > 0) & (order < ncl)
        rows = cl_lo + order[valid_m]
        np.maximum.at(full, rows, pm[valid_m])
        valid_b = (cm["cnt"][:NCL] > 0) & (np.arange(NCL) < ncl)
        rows = cl_lo + np.nonzero(valid_b)[0]
        np.maximum.at(full, rows, pb[valid_b])
    full[gcounts == 0] = -np.inf
    return full


# revision 15
# speedup vs baseline: 1.0800x; 1.0800x over previous
"""Trainium2 Bass kernel for sorted segment_max (ClusterPool).

Problem: features [2M, 128] f32, segment_ids [2M] sorted int, num_clusters
10000 -> output [10000, 128] = per-cluster max over rows.

Strategy (8 NeuronCores, SPMD):
  - Shard rows: core c gets rows [c*250k, (c+1)*250k).  Sorted ids => each
    core covers a contiguous cluster range (~1252 clusters), padded to 1280
    local clusters = 10 batches x 128.
  - MAIN pass (aligned blocks): cluster rows are covered by the 8-row-aligned
    4KB blocks fully inside [s, e).  dma_gather with elem_step=1024 makes the
    int16 index a BLOCK index (RPC/8 = 31250 < 2^15), so the whole core is
    addressable with no window.  That allows batches to be ANY 128 clusters:
    we sort clusters by block count so every batch has near-uniform count,
    cutting padding from ~1.26x to ~1.03x.  Per batch of 128 clusters
    (cluster p on partition p), NH gather calls of <=8 blocks/cluster;
    tensor_reduce max over rows -> [128, 128]; tensor_tensor-combine; store.
  - BOUNDARY pass: the <=14 unaligned edge rows per cluster (head [s, 8*ceil(
    s/8)), tail [8*floor(e/8), e); all rows if no full block, provably <=14)
    are gathered row-granular (elem 512B) in CONSECUTIVE cluster batches
    (which fit the 32768-row int16 window), reduced to [128, 128], stored
    separately.
  - Host combines main partials (sorted order), boundary partials, across the
    8 cores with np.maximum; empty clusters -> -inf.
"""

import os
import sys

import numpy as np

sys.path.insert(0, "/opt/trn_rl_repo")

N_POINTS = 2_000_000
D = 128
N_CLUSTERS = 10_000
N_CORES = 8
RPC = N_POINTS // N_CORES  # rows per core
NBLKS = RPC // 8  # 8-row blocks per core (31250 < 2^15)
NCL = 1280  # padded local clusters per core
NBATCH = NCL // 128
WINDOW = 32768  # row window for the boundary gather
ELEM = 1024  # floats per main-gather element (8 rows)
NB = 14  # boundary row slots per cluster (max possible)
THMAX = 8  # max blocks per cluster per gather call

_last_results = None  # BassKernelResults of the most recent run (for test.py)


def _apply_drain_patch():
    """walrus TPB_CTRL supports a single sync wait; TileContext's tail drain
    accumulates one wait per outstanding proc.  Split them across NOPs."""
    import concourse.mybir as mybir
    import concourse.tile as tile
    from concourse.vector_clock import ScopedClock

    if getattr(tile.TileContext, "_drain_patched", False):
        return

    def _patched(self, tick_clock, wait_clock):
        nc = self.nc
        nop = nc.sync.nop(nofuse=True, hint="tail_drain_waits")
        wait_clock.add_sem_waits(nop.ins, ScopedClock({None: tick_clock.global_clock}))
        si = nop.ins.sync_info
        waits = list(si.on_wait) if si is not None and si.on_wait else []
        if len(waits) > 1:
            si.on_wait = waits[:1]
            for i in range(1, len(waits)):
                extra = nc.sync.nop(nofuse=True, hint=f"tail_drain_waits_{i}")
                if extra.ins.sync_info is None:
                    extra.ins.sync_info = mybir.SyncInfo(
                        on_wait=waits[i : i + 1], on_update=[]
                    )
                else:
                    extra.ins.sync_info.on_wait = waits[i : i + 1]
        nc.sync.drain()
        nc.all_engine_barrier()
        assert self.sems is not None
        popped = nc._tile_sem_poison_stack.pop()
        assert popped is self._sem_poison
        nc.clear_and_free_semaphores(list(self.sems.allocated().values()))
        nc.all_engine_barrier()

    tile.TileContext._drain_and_barrier = _patched
    tile.TileContext._drain_patched = True


def _build_program(tms, bwindows):
    """Build the SPMD Bass program.

    tms[m]      = main-pass blocks per cluster for sorted batch m
    bwindows[m] = boundary-pass window base row for consecutive batch m"""
    import concourse.bacc as bacc
    import concourse.mybir as mybir
    import concourse.tile as tile
    from concourse.bass import AP

    _apply_drain_patch()

    CM = max(tms) * 8  # main idx cols per batch table

    nc = bacc.Bacc(None, num_swdge_queues=4)
    f_in = nc.dram_tensor("features", [RPC, D], mybir.dt.float32, kind="ExternalInput")
    mi_in = nc.dram_tensor(
        "midx", [NBATCH, 128, CM], mybir.dt.int16, kind="ExternalInput"
    )
    bi_in = nc.dram_tensor(
        "bidx", [NBATCH, 128, NB * 8], mybir.dt.int16, kind="ExternalInput"
    )
    pm_out = nc.dram_tensor(
        "pmain", [NCL, D], mybir.dt.float32, kind="ExternalOutput"
    )
    pb_out = nc.dram_tensor(
        "pbnd", [NCL, D], mybir.dt.float32, kind="ExternalOutput"
    )

    fbase = f_in[:, :]
    blocks = fbase.rearrange("(a b) d -> a (b d)", b=8)  # [31250, 1024]
    gq = 0
    with tile.TileContext(nc) as tc:
        with (
            tc.tile_pool(name="gp", bufs=5) as gp,
            tc.tile_pool(name="bp", bufs=2) as bp,
            tc.tile_pool(name="ip", bufs=3) as ip,
            tc.tile_pool(name="sp", bufs=2) as sp,
        ):
            def boundary(m):
                # ---- boundary pass: consecutive batch m, edge rows -------
                w = bwindows[m]
                win = AP(fbase.tensor, w * D, [[D, WINDOW], [1, D]])
                bt = ip.tile([128, NB * 8], mybir.dt.int16, tag="bidx")
                nc.sync.dma_start(out=bt[:], in_=bi_in[m])
                bg = bp.tile([128, NB * D], mybir.dt.float32, tag="bg")
                nonlocal gq
                nc.gpsimd.dma_gather(
                    out_ap=bg[:].rearrange("p (t e) -> p t e", e=D),
                    in_ap=win,
                    idxs_ap=bt[:],
                    num_idxs=NB * 128,
                    num_idxs_reg=NB * 128,
                    elem_size=D,
                    queue_num=gq % 4,
                    single_packet=False,
                )
                gq += 1
                br = sp.tile([128, D], mybir.dt.float32, tag="bred")
                nc.vector.tensor_reduce(
                    out=br[:],
                    in_=bg[:].rearrange("p (t d) -> p d t", d=D),
                    axis=mybir.AxisListType.X,
                    op=mybir.AluOpType.max,
                )
                nc.sync.dma_start(out=pb_out[m * 128 : (m + 1) * 128, :], in_=br[:])

            for m in range(NBATCH):
                # ---- main pass: sorted batch m, aligned blocks -----------
                T = tms[m]
                it = ip.tile([128, CM], mybir.dt.int16, tag="midx")
                nc.sync.dma_start(out=it[:], in_=mi_in[m])
                # split the first batch's first call into a small primer so
                # the first reduce starts as early as possible
                splits = list(range(0, T, THMAX))
                chunks = [
                    (c0, min(THMAX, T - c0)) for c0 in splits
                ]
                if m == 0 and chunks[0][1] > 2:
                    chunks = [(0, 2), (2, chunks[0][1] - 2)] + chunks[1:]
                acc = None
                for c0, TH in chunks:
                    g = gp.tile([128, THMAX * ELEM], mybir.dt.float32, tag="gath")
                    nc.gpsimd.dma_gather(
                        out_ap=g[:, : TH * ELEM].rearrange(
                            "p (t e) -> p t e", e=ELEM
                        ),
                        in_ap=blocks,
                        idxs_ap=it[:, c0 * 8 : (c0 + TH) * 8],
                        num_idxs=TH * 128,
                        num_idxs_reg=TH * 128,
                        elem_size=ELEM,
                        queue_num=gq % 4,
                        single_packet=False,
                    )
                    gq += 1
                    r = sp.tile([128, D], mybir.dt.float32, tag=f"red{c0 % (2 * THMAX)}")
                    nc.vector.tensor_reduce(
                        out=r[:],
                        in_=g[:, : TH * ELEM].rearrange("p (t d) -> p d t", d=D),
                        axis=mybir.AxisListType.X,
                        op=mybir.AluOpType.max,
                    )
                    if acc is None:
                        acc = r
                    else:
                        nc.vector.tensor_tensor(
                            out=acc[:], in0=acc[:], in1=r[:],
                            op=mybir.AluOpType.max,
                        )
                nc.sync.dma_start(out=pm_out[m * 128 : (m + 1) * 128, :], in_=acc[:])
                boundary(m)

    if not nc.is_finalized():
        nc.finalize()
    return nc


def kernel(features, segment_ids, num_clusters):
    global _last_results
    from concourse.bass_utils import run_bass_kernel_spmd

    features = np.ascontiguousarray(np.asarray(features, dtype=np.float32))
    ids = np.asarray(segment_ids).astype(np.int64)
    nclusters = int(num_clusters)
    assert features.shape == (N_POINTS, D), features.shape
    assert ids.shape == (N_POINTS,)
    assert nclusters == N_CLUSTERS

    # --- host cluster metadata -------------------------------------------
    gstart = np.searchsorted(ids, np.arange(nclusters), side="left")
    gend = np.searchsorted(ids, np.arange(nclusters) + 1, side="left")
    gcounts = gend - gstart

    core_meta = []  # per core dict
    for c in range(N_CORES):
        r0, r1 = c * RPC, (c + 1) * RPC
        cl_lo, cl_hi = int(ids[r0]), int(ids[r1 - 1])
        ncl = cl_hi - cl_lo + 1
        assert ncl <= NCL, f"core {c}: {ncl} local clusters > {NCL}"
        s = np.clip(gstart[cl_lo : cl_hi + 1], r0, r1) - r0
        e = np.clip(gend[cl_lo : cl_hi + 1], r0, r1) - r0
        s_pad = np.zeros(NCL, dtype=np.int64)
        e_pad = np.zeros(NCL, dtype=np.int64)
        s_pad[:ncl] = s
        e_pad[:ncl] = e
        cnt = e_pad - s_pad
        a0 = (s_pad + 7) // 8  # first aligned block fully inside
        a1 = e_pad // 8  # one past last aligned block fully inside
        nb = np.maximum(a1 - a0, 0)
        nb[cnt == 0] = 0
        # cnt >= 15 guarantees nb >= 1; nb == 0 => cnt <= 14 (fits NB slots)
        assert int(cnt[(nb == 0)].max(initial=0)) <= NB
        order = np.argsort(nb, kind="stable")  # sorted batches for main pass
        core_meta.append(
            dict(cl_lo=cl_lo, ncl=ncl, s=s_pad, e=e_pad, cnt=cnt,
                 a0=a0, nb=nb, order=order)
        )

    # Main-pass per-batch block counts (shared across cores).
    tms = []
    for m in range(NBATCH):
        T = 1
        for cm in core_meta:
            sel = cm["order"][m * 128 : (m + 1) * 128]
            T = max(T, int(cm["nb"][sel].max()))
        tms.append(T)
    CM = max(tms) * 8

    # Boundary windows (consecutive batches, shared across cores).
    bwindows = []
    for m in range(NBATCH):
        jj = slice(m * 128, (m + 1) * 128)
        wmin = RPC
        for cm in core_meta:
            act = cm["cnt"][jj] > 0
            if act.any():
                wmin = min(wmin, int(cm["s"][jj][act].min()))
        w = max(0, min(wmin, RPC - WINDOW))
        bwindows.append(w)

    def wrap_calls(V, T):
        """V[j] with j = t*128+p for t in [0,T) -> wrapped int16 [128, T*8]."""
        tab = V.reshape(T * 8, 16).T  # [16, T*8]; col m covers V[m*16:(m+1)*16]
        return np.tile(tab, (8, 1))

    # --- main-pass tables -------------------------------------------------
    midx_all, bidx_all = [], []
    for cm in core_meta:
        g = np.zeros((NBATCH, 128, CM), dtype=np.int16)
        for m in range(NBATCH):
            T = tms[m]
            sel = cm["order"][m * 128 : (m + 1) * 128]
            a0 = cm["a0"][sel]
            nb = cm["nb"][sel]
            t = np.arange(T)[None, :]
            blk = a0[:, None] + t
            last = np.where(nb > 0, a0 + nb - 1, 0)
            blk = np.where(t >= nb[:, None], last[:, None], blk)
            blk = np.where((nb[:, None] > 0), blk, 0)
            assert blk.min() >= 0 and blk.max() < NBLKS
            V = blk.astype(np.int16).T.reshape(-1)  # j = t*128 + p
            g[m, :, : T * 8] = wrap_calls(V, T)
        midx_all.append(g)

        # boundary tables: consecutive batches
        b = np.zeros((NBATCH, 128, NB * 8), dtype=np.int16)
        for m in range(NBATCH):
            jj = slice(m * 128, (m + 1) * 128)
            s = cm["s"][jj]
            e = cm["e"][jj]
            cnt = cm["cnt"][jj]
            a0 = cm["a0"][jj]
            nb = cm["nb"][jj]
            w = bwindows[m]
            rows = np.zeros((128, NB), dtype=np.int64)
            for p in range(128):
                if cnt[p] == 0:
                    rows[p] = w
                    continue
                if nb[p] > 0:
                    head = np.arange(s[p], 8 * a0[p])
                    tail = np.arange(8 * (a0[p] + nb[p]), e[p])
                    rl = np.concatenate([head, tail])
                else:
                    rl = np.arange(s[p], e[p])
                nrl = len(rl)
                assert nrl <= NB
                rows[p, :nrl] = rl
                rows[p, nrl:] = s[p]  # repeat first row
            rel = rows - w
            assert rel.min() >= 0 and rel.max() < WINDOW, (m, rel.min(), rel.max())
            V = rel.astype(np.int16).T.reshape(-1)  # j = t*128+p, t in [0, NB)
            b[m] = wrap_calls(V, NB)
        bidx_all.append(b)

    # --- build + run ------------------------------------------------------
    nc = _build_program(tms, bwindows)
    in_maps = [
        {
            "features": features[c * RPC : (c + 1) * RPC],
            "midx": midx_all[c],
            "bidx": bidx_all[c],
        }
        for c in range(N_CORES)
    ]
    res = run_bass_kernel_spmd(nc, in_maps, list(range(N_CORES)))
    _last_results = res

    # --- host combine -----------------------------------------------------
    full = np.full((nclusters, D), -np.inf, dtype=np.float32)
    for c in range(N_CORES):
        cm = core_meta[c]
        cl_lo, ncl = cm["cl_lo"], cm["ncl"]
        pm = res.results[c]["pmain"]  # [NCL, D] in sorted order
        pb = res.results[c]["pbnd"]  # [NCL, D] in consecutive order
        order = cm["order"]
        nb_sorted = cm["nb"][order]
        valid_m = (nb_sorted > 0) & (order < ncl)
        rows = cl_lo + order[valid_m]
        np.maximum.at(full, rows, pm[valid_m])
        valid_b = (cm["cnt"][:NCL] > 0) & (np.arange(NCL) < ncl)
        rows = cl_lo + np.nonzero(valid_b)[0]
        np.maximum.at(full, rows, pb[valid_b])
    full[gcounts == 0] = -np.inf
    return full
